# revision 27
# baseline (speedup 1.0000x reference)
"""Multi-head attention (B=4, T=2048, C=1024, H=16, causal) on 8 TRN2 cores.

Sharding: core c -> batch b = c//2, head-half h2 = c%2 (8 heads / core).
v4: host pre-transposes x (kills on-device xbar transposes), attention
runs tq-outer/pair-inner starting right after the strip-0 K/Q
projections, and the remaining projections + per-strip output
projection are interleaved into the attention phase as PE filler so
the exp-paced (ACT-saturated) window never idles the PE.
Each core emits its partial out^T over full T; the host sums the pair
during unshard (bo passed as bo/2).
"""

import sys

sys.path.insert(0, "/opt/trn_rl_repo")

import numpy as np

import concourse.bacc as bacc
import concourse.bass as bass
import concourse.mybir as mybir
import concourse.tile as tile
from concourse.bass_utils import run_bass_kernel_spmd

F32 = mybir.dt.float32
F32R = mybir.dt.float32r
BF = mybir.dt.bfloat16
AF = mybir.ActivationFunctionType

P = 128          # partitions
T = 2048         # sequence length
C = 1024         # model dim
FS = 512         # per-core feature slice (8 heads x 64)
NH = 8           # heads per core
HD = 64          # head dim
SCALE = 0.125    # 1/sqrt(64)
NCORES = 8

NTQ = 4          # T / 512 query tiles
NFB = 4          # FS / 128 feature blocks
NCB = 8          # C / 128 contraction blocks
NTT = 16         # T / 128 key tiles


def build_program():
    nc = bacc.Bacc(num_devices=NCORES)

    # host-pre-transposed inputs: x*[c, t] = x[t, c]
    xq = nc.declare_dram_parameter("xq", [C, T], BF, isOutput=False)
    xk = nc.declare_dram_parameter("xk", [C, T], BF, isOutput=False)
    xv = nc.declare_dram_parameter("xv", [C, T], BF, isOutput=False)
    # wq/wk[p, fb, cb, j] = W[128*cb + p, 512*h2 + 128*fb + j]
    wq = nc.declare_dram_parameter("wq", [P, NFB, NCB, P], BF, isOutput=False)
    wk = nc.declare_dram_parameter("wk", [P, NFB, NCB, P], BF, isOutput=False)
    wv = nc.declare_dram_parameter("wv", [C, FS], BF, isOutput=False)
    # wo[p, cc, fc, j] = Wo[fsl, :][128*fc + p, 128*cc + j]
    wo = nc.declare_dram_parameter("wo", [P, NCB, NFB, P], BF, isOutput=False)
    bq = nc.declare_dram_parameter("bq", [P, NFB], F32, isOutput=False)
    bk = nc.declare_dram_parameter("bk", [P, NFB], F32, isOutput=False)
    bv = nc.declare_dram_parameter("bv", [1, FS], F32, isOutput=False)
    bo = nc.declare_dram_parameter("bo", [P, NCB], F32, isOutput=False)
    # maskx[p, u] = 1.0 iff u >= p + 384; diag tile di mask slice at 384-128*di
    maskx = nc.declare_dram_parameter("maskx", [P, 896], BF, isOutput=False)
    out = nc.declare_dram_parameter("out", [C, T], BF, isOutput=True)

    with tile.TileContext(nc) as tc:
        import contextlib

        with contextlib.ExitStack() as ctx:
            consts = ctx.enter_context(tc.tile_pool(name="consts", bufs=1))
            xt_pool = ctx.enter_context(tc.tile_pool(name="xt", bufs=16))
            wqk_pool = ctx.enter_context(tc.tile_pool(name="wqk", bufs=1))
            wv_pool = ctx.enter_context(tc.tile_pool(name="wvp", bufs=1))
            wo_pool = ctx.enter_context(tc.tile_pool(name="wop", bufs=1))
            kt_pool = ctx.enter_context(tc.tile_pool(name="ktp", bufs=1))
            qt_pool = ctx.enter_context(tc.tile_pool(name="qtp", bufs=1))
            v_pool = ctx.enter_context(tc.tile_pool(name="vp", bufs=1))
            y_pool = ctx.enter_context(tc.tile_pool(name="yp", bufs=1))
            ex_pool = ctx.enter_context(tc.tile_pool(name="exp", bufs=10))
            rc_pool = ctx.enter_context(tc.tile_pool(name="rcp", bufs=2))
            rb_pool = ctx.enter_context(tc.tile_pool(name="rbp", bufs=2))
            yr_pool = ctx.enter_context(tc.tile_pool(name="yrp", bufs=2))
            ob_pool = ctx.enter_context(tc.tile_pool(name="ob", bufs=2))
            psS = ctx.enter_context(tc.tile_pool(name="psS", bufs=3, space="PSUM"))
            psY = ctx.enter_context(tc.tile_pool(name="psY", bufs=2, space="PSUM"))
            dram = ctx.enter_context(tc.tile_pool(name="dram", bufs=2,
                                                  space="DRAM"))

            # ---- DMAs in PE need-order, column-split so the first-half
            # tiles (all that phase-1/2 chains touch) land early
            bv_sb = consts.tile([P, FS], F32, tag="bv", name="bv_sb")
            nc.sync.dma_start(bv_sb[:], bv[:].to_broadcast((P, FS)))
            wvb = wv_pool.tile([P, NCB * FS], BF, tag="wv", name="wvb")
            nc.sync.dma_start(
                wvb[:].rearrange("p (cb f) -> p cb f", f=FS),
                wv[:].rearrange("(cb p) f -> p cb f", p=P),
            )
            wv_sb = [wvb[:, FS * cb : FS * (cb + 1)] for cb in range(NCB)]

            xtv = [xt_pool.tile([P, T], BF, tag="xt", name="xtv_t")
                   for cb in range(NCB)]
            for half in range(2):
                tsl = slice(1024 * half, 1024 * (half + 1))
                for cb in range(NCB):
                    nc.sync.dma_start(xtv[cb][:, tsl],
                                      xv[P * cb : P * (cb + 1), tsl])

            wkb = wqk_pool.tile([P, NFB * NCB * P], BF, tag="wkb", name="wkb")
            nc.sync.dma_start(
                wkb[:].rearrange("p (fb cb j) -> p fb cb j", cb=NCB, j=P), wk[:]
            )
            def wk_sb(fb, cb):
                o = NCB * P * fb + P * cb
                return wkb[:, o : o + P]

            ba_t = consts.tile([P, 2 * NFB + NCB], F32, tag="ba", name="ba_t")
            nc.sync.dma_start(ba_t[:, 0:NFB], bq[:])
            nc.sync.dma_start(ba_t[:, NFB : 2 * NFB], bk[:])
            nc.sync.dma_start(ba_t[:, 2 * NFB :], bo[:])
            bq_sb = [ba_t[:, i : i + 1] for i in range(NFB)]
            bk_sb = [ba_t[:, NFB + i : NFB + i + 1] for i in range(NFB)]
            bo_sb = [ba_t[:, 2 * NFB + i : 2 * NFB + i + 1] for i in range(NCB)]
            ones_sb = consts.tile([1, HD], BF, tag="ones", name="ones_sb")
            nc.gpsimd.memset(ones_sb[:], 1.0)

            xtk = [xt_pool.tile([P, T], BF, tag="xt", name="xtk_t")
                   for cb in range(NCB)]
            for cb in range(NCB):
                nc.sync.dma_start(xtk[cb][:, 0:1024],
                                  xk[P * cb : P * (cb + 1), 0:1024])

            wqb = wqk_pool.tile([P, NFB * NCB * P], BF, tag="wqb", name="wqb")
            nc.sync.dma_start(
                wqb[:].rearrange("p (fb cb j) -> p fb cb j", cb=NCB, j=P), wq[:]
            )
            def wq_sb(fb, cb):
                o = NCB * P * fb + P * cb
                return wqb[:, o : o + P]

            xtq = [xt_pool.tile([P, T], BF, tag="xt", name="xtq_t")
                   for cb in range(NCB)]
            for cb in range(NCB):
                nc.sync.dma_start(xtq[cb][:, 0:1024],
                                  xq[P * cb : P * (cb + 1), 0:1024])
            for cb in range(NCB):
                nc.sync.dma_start(xtk[cb][:, 1024:2048],
                                  xk[P * cb : P * (cb + 1), 1024:2048])
            for cb in range(NCB):
                nc.sync.dma_start(xtq[cb][:, 1024:2048],
                                  xq[P * cb : P * (cb + 1), 1024:2048])

            mx_sb = consts.tile([P, 896], BF, tag="maskx", name="mx_sb")
            nc.sync.dma_start(mx_sb[:], maskx[:])
            wob = wo_pool.tile([P, NCB * NFB * P], BF, tag="wo", name="wob")
            nc.sync.dma_start(
                wob[:].rearrange("p (cc fc j) -> p cc fc j", fc=NFB, j=P), wo[:]
            )
            def wo_sb(cc, fc):
                o = NFB * P * cc + P * fc
                return wob[:, o : o + P]

            # ---- persistent attention operands
            KT = [kt_pool.tile([P, T], BF, tag=f"kt{i}", name=f"kt{i}")
                  for i in range(NFB)]
            QT = [qt_pool.tile([P, T], BF, tag=f"qt{i}", name=f"qt{i}")
                  for i in range(NFB)]
            # V tiles carry an inline ones column per head: [v_h | 1] x 8
            VSB = [v_pool.tile([P, NH * (HD + 1)], BF, tag=f"v{i}", name=f"v{i}")
                   for i in range(NTT)]
            # Y stays resident in SBUF (f-major, head h rows [64h%128] of fb=h//2)
            YSB = [y_pool.tile([P, T], BF, tag=f"y{i}", name=f"y{i}")
                   for i in range(NFB)]

            # ==============  V projection (cb-major in 3-chain groups so the
            # PE has work while the xv tiles stream in)  ==============
            for tis in ([0, 1, 2], [3, 4, 5], [6, 7],
                        [8, 9, 10], [11, 12, 13], [14, 15]):
                pvs = {}
                for ti in tis:
                    pvs[ti] = psS.tile([P, FS], F32, tag="psS", name="pv")
                for cb in range(NCB):
                    for ti in tis:
                        nc.tensor.matmul(
                            pvs[ti][:], xtv[cb][:, P * ti : P * (ti + 1)],
                            wv_sb[cb],
                            start=(cb == 0), stop=(cb == NCB - 1),
                        )
                for ti in tis:
                    vt = VSB[ti]
                    v3 = vt[:].rearrange("p (h x) -> p h x", x=HD + 1)
                    nc.vector.tensor_add(
                        v3[:, :, 0:HD],
                        pvs[ti][:].rearrange("p (h d) -> p h d", d=HD),
                        bv_sb[:].rearrange("p (h d) -> p h d", d=HD),
                    )
                    nc.gpsimd.memset(v3[:, :, HD], 1.0)

            # ---- one K/Q projection chain: 1024 t-cols for one fb
            def proj_chain(fb, tqp, xt_src, w_sb, bias_sb, OUT):
                pp = psS.tile([P, 1024], F32, tag="psS", name="pp")
                for cb in range(NCB):
                    for u in range(2):
                        tq = 2 * tqp + u
                        nc.tensor.matmul(
                            pp[:, 512 * u : 512 * (u + 1)],
                            w_sb(fb, cb),
                            xt_src[cb][:, 512 * tq : 512 * (tq + 1)],
                            start=(cb == 0), stop=(cb == NCB - 1),
                        )
                for u in range(2):
                    tq = 2 * tqp + u
                    nc.vector.tensor_scalar_add(
                        OUT[fb][:, 512 * tq : 512 * (tq + 1)],
                        pp[:, 512 * u : 512 * (u + 1)],
                        bias_sb[fb],
                    )

            # half-size (512-col) projection chain: finer-grained filler
            def proj_chain_h(fb, tq, xt_src, w_sb, bias_sb, OUT):
                pp = psS.tile([P, 1024], F32, tag="psS", name="pph")
                for cb in range(NCB):
                    nc.tensor.matmul(
                        pp[:, 0:512],
                        w_sb(fb, cb),
                        xt_src[cb][:, 512 * tq : 512 * (tq + 1)],
                        start=(cb == 0), stop=(cb == NCB - 1),
                    )
                nc.vector.tensor_scalar_add(
                    OUT[fb][:, 512 * tq : 512 * (tq + 1)],
                    pp[:, 0:512],
                    bias_sb[fb],
                )

            # strip-0/1 K then Q projections up-front; tqp=1 chains become
            # attention-phase filler
            for fb in range(NFB):
                proj_chain(fb, 0, xtk, wk_sb, bk_sb, KT)
            for fb in range(NFB):
                proj_chain(fb, 0, xtq, wq_sb, bq_sb, QT)

            # ---- output projection chunk: 2 cc rows x one 512-col t-strip
            def outproj_chunk(tq, ccp, act_bias=False):
                pso = psS.tile([P, 1024], F32, tag="psS", name="pso")
                for fc in range(NFB):
                    for w in range(2):
                        cc = 2 * ccp + w
                        nc.tensor.matmul(
                            pso[:, 512 * w : 512 * (w + 1)],
                            wo_sb(cc, fc),
                            YSB[fc][:, 512 * tq : 512 * (tq + 1)],
                            start=(fc == 0), stop=(fc == NFB - 1),
                        )
                # host passes bo/2 so the host-side pair sum restores bo
                osb = ob_pool.tile([P, 1024], BF, tag="ob", name="osb")
                for w in range(2):
                    cc = 2 * ccp + w
                    if act_bias:
                        # tail strips: ACT is done with exps, so bias there
                        # keeps the (finalize-busy) DVE off this path
                        nc.scalar.add(
                            osb[:, 512 * w : 512 * (w + 1)],
                            pso[:, 512 * w : 512 * (w + 1)], bo_sb[cc])
                    else:
                        nc.vector.tensor_scalar_add(
                            osb[:, 512 * w : 512 * (w + 1)],
                            pso[:, 512 * w : 512 * (w + 1)], bo_sb[cc])
                    nc.sync.dma_start(
                        out[P * cc : P * (cc + 1),
                            512 * tq : 512 * (tq + 1)],
                        osb[:, 512 * w : 512 * (w + 1)],
                    )

            # ---- PE filler: K/Q strip-2/3 projection chunks are emitted
            # just-in-time per block; finished strips' output projections
            # queue up as generic filler.
            def k_chunk(tq, fb):
                proj_chain_h(fb, tq, xtk, wk_sb, bk_sb, KT)

            def q_chunk(tq, fb):
                proj_chain_h(fb, tq, xtq, wq_sb, bq_sb, QT)

            op_q = []

            def feed(n=1):
                for _ in range(min(n, len(op_q))):
                    op_q.pop(0)()

            # ================  attention: tq-outer, pair-inner  ================
            def attention_block(pair, tq, on_group, pe_bcast=False):
                ntk = 4 * (tq + 1)
                ngrp = ntk // 2
                qsl = slice(512 * tq, 512 * (tq + 1))
                psy = [
                    psY.tile([HD + 1, 512], F32, tag="psY", name=f"psy{s}")
                    for s in range(2)
                ]
                # software pipeline (depth 4): emit scores(g)+exp(g) ahead
                # of attV(g-4) so the PE's in-order stream never waits on
                # the ACT exp of the group it is about to consume.
                DEPTH = 4
                exq = {}
                for g in range(ngrp + DEPTH):
                    if g < ngrp:
                        for s in range(2):
                            rows = slice(64 * s, 64 * (s + 1))
                            ps = psS.tile([P, 1024], F32, tag="psS",
                                          name="ps_s")
                            for j in range(2):
                                tk = 2 * g + j
                                # diag tiles: only q >= 128*di is live
                                o_ = P * max(tk - 4 * tq, 0)
                                # 64-row array tiling: head s=0 in rows
                                # 0-63 (T0), s=1 in rows 64-127 (T8) so
                                # the two heads' LDW+MM run concurrently
                                nc.tensor.matmul(
                                    ps[:, 512 * j + o_ : 512 * (j + 1)],
                                    KT[pair][rows, P * tk : P * (tk + 1)],
                                    QT[pair][rows,
                                             512 * tq + o_ :
                                             512 * (tq + 1)],
                                    start=True, stop=True,
                                    tile_position=(64 * s, 0),
                                )
                            # exp; cols below the live offset hold garbage
                            # that no attV matmul reads. For the deep
                            # diagonal group, skip the dead columns.
                            ex = ex_pool.tile([P, 1024], BF, tag="ex",
                                              name="ex")
                            di0 = 2 * g - 4 * tq
                            if di0 == 2:
                                nc.scalar.activation(
                                    ex[:, 256:512], ps[:, 256:512],
                                    AF.Exp, scale=SCALE)
                                nc.scalar.activation(
                                    ex[:, 896:1024], ps[:, 896:1024],
                                    AF.Exp, scale=SCALE)
                            else:
                                nc.scalar.activation(ex[:], ps[:], AF.Exp,
                                                     scale=SCALE)
                            for j in range(2):
                                di = 2 * g + j - 4 * tq
                                if di >= 0:
                                    # triangular boundary block only
                                    o_ = 512 * j + P * di
                                    nc.vector.tensor_mul(
                                        ex[:, o_ : o_ + P],
                                        ex[:, o_ : o_ + P],
                                        mx_sb[:, 384:512],
                                    )
                            exq[(g, s)] = ex
                        on_group(g)
                    gd = g - DEPTH
                    if gd < 0:
                        continue
                    for s in range(2):
                        h = 2 * pair + s
                        vsl0 = (HD + 1) * h
                        ex = exq.pop((gd, s))
                        for j in range(2):
                            tk = 2 * gd + j
                            o_ = P * max(tk - 4 * tq, 0)
                            nc.tensor.matmul(
                                psy[s][:, o_:],
                                VSB[tk][:, vsl0 : vsl0 + HD + 1],
                                ex[:, 512 * j + o_ : 512 * (j + 1)],
                                start=(tk == 0), stop=(tk == ntk - 1),
                            )
                if pe_bcast:
                    # tail block: minimize finalize latency. Reciprocal
                    # straight off PSUM, y staged via the (now idle) ACT,
                    # and the partition-broadcast via a rank-1 ones matmul
                    # on the (otherwise idle) PE instead of the DRAM trip.
                    for s in range(2):
                        den = rc_pool.tile([1, 512], F32, tag="den",
                                           name="den")
                        nc.vector.tensor_copy(den[:], psy[s][HD : HD + 1, :])
                        rc = rc_pool.tile([1, 512], F32, tag="rc", name="rc")
                        nc.vector.reciprocal_approx_fast(rc[:], den[:])
                        rcb = rc_pool.tile([1, 512], BF, tag="rcb", name="rcb")
                        nc.vector.tensor_copy(rcb[:], rc[:])
                        yraw = yr_pool.tile([HD, 512], BF, tag="yr",
                                            name="yr")
                        nc.scalar.copy(yraw[:], psy[s][0:HD, :])
                        rbp = psS.tile([P, 1024], F32, tag="psS", name="rbp")
                        nc.tensor.matmul(rbp[0:HD, 0:512], ones_sb[:],
                                         rcb[:], start=True, stop=True)
                        nc.vector.tensor_mul(
                            YSB[pair][64 * s : 64 * (s + 1), qsl],
                            yraw[:], rbp[0:HD, 0:512],
                        )
                    return
                for s in range(2):
                    # stage y and denominator out of PSUM promptly so the
                    # psY slot frees for the next block
                    yraw = yr_pool.tile([HD, 512], BF, tag="yr", name="yr")
                    nc.vector.tensor_copy(yraw[:], psy[s][0:HD, :])
                    den = rc_pool.tile([1, 512], F32, tag="den", name="den")
                    nc.vector.tensor_copy(den[:], psy[s][HD : HD + 1, :])
                    rc = rc_pool.tile([1, 512], F32, tag="rc", name="rc")
                    nc.vector.reciprocal_approx_fast(rc[:], den[:])
                    # broadcast across partitions via a DRAM round-trip
                    # (partition-stride-0 DMA reads require a DRAM source);
                    # keeps the PE stream out of the finalize entirely
                    rcd = dram.tile([1, 512], F32, tag="rcd", name="rcd")
                    nc.sync.dma_start(rcd[:], rc[:])
                    rb = rb_pool.tile([HD, 512], F32, tag="rb", name="rb")
                    nc.sync.dma_start(rb[:], rcd[:].to_broadcast((HD, 512)))
                    nc.vector.tensor_mul(
                        YSB[pair][64 * s : 64 * (s + 1), qsl],
                        yraw[:], rb[:],
                    )

            # Strip-interleaved block order: heavy strips (2,3) alternate
            # with light strips (0,1) so the ACT exp stream stays fed from
            # the first scores to the last — otherwise the final strip's
            # exps pace the tail. Q cols for a heavy block are emitted just
            # before it, K cols (needed only from group 4) inside it.
            BLOCK_ORDER = [(2, 0), (0, 0), (2, 1), (0, 1),
                           (2, 2), (0, 2), (2, 3), (0, 3),
                           (3, 0), (1, 0), (3, 1), (1, 1),
                           (3, 2), (1, 2), (3, 3), (1, 3)]
            strip_left = {tq: NFB for tq in range(NTQ)}
            for idx, (tq, pair) in enumerate(BLOCK_ORDER):
                if tq >= 2:
                    q_chunk(tq, pair)
                ngrp = 2 * (tq + 1)
                last = (idx == len(BLOCK_ORDER) - 1)

                def on_group(g, tq=tq, pair=pair, ngrp=ngrp, idx=idx,
                             last=last):
                    if tq >= 2 and g == 0:
                        k_chunk(tq, pair)
                    if last:
                        feed(1)
                    elif idx >= 8 and g in (1, ngrp // 2 + 1):
                        feed(1)
                attention_block(pair, tq, on_group, pe_bcast=last)
                strip_left[tq] -= 1
                if strip_left[tq] == 0 and not last:
                    for ccp in range(NCB // 2):
                        op_q.append(
                            lambda tq=tq, ccp=ccp: outproj_chunk(tq, ccp))

            # drain leftover filler + last-finished strip's output projection
            feed(len(op_q))
            for ccp in range(NCB // 2):
                outproj_chunk(BLOCK_ORDER[-1][0], ccp, act_bias=True)

    nc.compile()
    return nc


_NC_CACHE = None


def _get_nc():
    global _NC_CACHE
    if _NC_CACHE is None:
        _NC_CACHE = build_program()
    return _NC_CACHE


def _host_consts():
    import ml_dtypes

    pgrid, ugrid = np.mgrid[0:P, 0:896]
    maskxv = (ugrid >= pgrid + 384).astype(ml_dtypes.bfloat16)
    return maskxv


def _w_qk_layout(w):
    # [p, fb, cb, j] = w[128*cb + p, 128*fb + j]
    return np.ascontiguousarray(
        w.reshape(NCB, P, NFB, P).transpose(1, 2, 0, 3))


def _w_o_layout(w):
    # [p, cc, fc, j] = w[128*fc + p, 128*cc + j]
    return np.ascontiguousarray(
        w.reshape(NFB, P, NCB, P).transpose(1, 2, 0, 3))


def _make_in_maps(inputs) -> list:
    import ml_dtypes

    BF16 = ml_dtypes.bfloat16

    def bf(a):
        return np.ascontiguousarray(np.asarray(a, dtype=np.float32)).astype(BF16)

    q = np.asarray(inputs["q"], dtype=np.float32)
    k = np.asarray(inputs["k"], dtype=np.float32)
    v = np.asarray(inputs["v"], dtype=np.float32)
    Wq = np.asarray(inputs["Wq"], dtype=np.float32)
    Wk = np.asarray(inputs["Wk"], dtype=np.float32)
    Wv = np.asarray(inputs["Wv"], dtype=np.float32)
    Wo = np.asarray(inputs["Wo"], dtype=np.float32)
    bq = np.asarray(inputs["bq"], dtype=np.float32)
    bk = np.asarray(inputs["bk"], dtype=np.float32)
    bv = np.asarray(inputs["bv"], dtype=np.float32)
    bo = np.asarray(inputs["bo"], dtype=np.float32)
    # mask is all-ones in this problem (causal handled in-kernel); ignored.

    maskxv = _host_consts()
    in_maps = []
    for c in range(NCORES):
        b, h2 = divmod(c, 2)
        fsl = slice(FS * h2, FS * (h2 + 1))
        in_maps.append({
            "xq": bf(q[b].T),
            "xk": bf(k[b].T),
            "xv": bf(v[b].T),
            "wq": _w_qk_layout(Wq[:, fsl]).astype(BF16),
            "wk": _w_qk_layout(Wk[:, fsl]).astype(BF16),
            "wv": bf(Wv[:, fsl]),
            "wo": _w_o_layout(Wo[fsl, :]).astype(BF16),
            "bq": np.ascontiguousarray(bq[fsl].reshape(NFB, P).T),
            "bk": np.ascontiguousarray(bk[fsl].reshape(NFB, P).T),
            "bv": np.ascontiguousarray(bv[fsl].reshape(1, FS)),
            "bo": np.ascontiguousarray((bo / 2.0).reshape(NCB, P).T),
            "maskx": maskxv,
        })
    return in_maps


def kernel(**inputs) -> np.ndarray:
    in_maps = _make_in_maps(inputs)
    nc = _get_nc()
    res = run_bass_kernel_spmd(nc, in_maps, list(range(NCORES)))

    full = np.empty((4, T, C), dtype=np.float32)
    for b in range(4):
        po = (res.results[2 * b]["out"].astype(np.float32)
              + res.results[2 * b + 1]["out"].astype(np.float32))
        full[b] = po.T
    return full


# revision 28
# speedup vs baseline: 1.0272x; 1.0272x over previous
"""Multi-head attention (B=4, T=2048, C=1024, H=16, causal) on 8 TRN2 cores.

Sharding: core c -> batch b = c//2, head-half h2 = c%2 (8 heads / core).
v4: host pre-transposes x (kills on-device xbar transposes), attention
runs tq-outer/pair-inner starting right after the strip-0 K/Q
projections, and the remaining projections + per-strip output
projection are interleaved into the attention phase as PE filler so
the exp-paced (ACT-saturated) window never idles the PE.
Each core emits its partial out^T over full T; the host sums the pair
during unshard (bo passed as bo/2).
"""

import sys

sys.path.insert(0, "/opt/trn_rl_repo")

import numpy as np

import concourse.bacc as bacc
import concourse.bass as bass
import concourse.mybir as mybir
import concourse.tile as tile
from concourse.bass_utils import run_bass_kernel_spmd

F32 = mybir.dt.float32
F32R = mybir.dt.float32r
BF = mybir.dt.bfloat16
AF = mybir.ActivationFunctionType

P = 128          # partitions
T = 2048         # sequence length
C = 1024         # model dim
FS = 512         # per-core feature slice (8 heads x 64)
NH = 8           # heads per core
HD = 64          # head dim
SCALE = 0.125    # 1/sqrt(64)
NCORES = 8

NTQ = 4          # T / 512 query tiles
NFB = 4          # FS / 128 feature blocks
NCB = 8          # C / 128 contraction blocks
NTT = 16         # T / 128 key tiles


def build_program():
    nc = bacc.Bacc(num_devices=NCORES)

    # host-pre-transposed inputs: x*[c, t] = x[t, c]
    xq = nc.declare_dram_parameter("xq", [C, T], BF, isOutput=False)
    xk = nc.declare_dram_parameter("xk", [C, T], BF, isOutput=False)
    xv = nc.declare_dram_parameter("xv", [C, T], BF, isOutput=False)
    # wq/wk[p, fb, cb, j] = W[128*cb + p, 512*h2 + 128*fb + j]
    wq = nc.declare_dram_parameter("wq", [P, NFB, NCB, P], BF, isOutput=False)
    wk = nc.declare_dram_parameter("wk", [P, NFB, NCB, P], BF, isOutput=False)
    wv = nc.declare_dram_parameter("wv", [C, FS], BF, isOutput=False)
    # wo[p, cc, fc, j] = Wo[fsl, :][128*fc + p, 128*cc + j]
    wo = nc.declare_dram_parameter("wo", [P, NCB, NFB, P], BF, isOutput=False)
    bq = nc.declare_dram_parameter("bq", [P, NFB], F32, isOutput=False)
    bk = nc.declare_dram_parameter("bk", [P, NFB], F32, isOutput=False)
    bv = nc.declare_dram_parameter("bv", [1, FS], F32, isOutput=False)
    bo = nc.declare_dram_parameter("bo", [P, NCB], F32, isOutput=False)
    # maskx[p, u] = 1.0 iff u >= p + 384; diag tile di mask slice at 384-128*di
    maskx = nc.declare_dram_parameter("maskx", [P, 896], BF, isOutput=False)
    out = nc.declare_dram_parameter("out", [C, T], BF, isOutput=True)

    with tile.TileContext(nc) as tc:
        import contextlib

        with contextlib.ExitStack() as ctx:
            consts = ctx.enter_context(tc.tile_pool(name="consts", bufs=1))
            xt_pool = ctx.enter_context(tc.tile_pool(name="xt", bufs=16))
            wqk_pool = ctx.enter_context(tc.tile_pool(name="wqk", bufs=1))
            wv_pool = ctx.enter_context(tc.tile_pool(name="wvp", bufs=1))
            wo_pool = ctx.enter_context(tc.tile_pool(name="wop", bufs=1))
            kt_pool = ctx.enter_context(tc.tile_pool(name="ktp", bufs=1))
            qt_pool = ctx.enter_context(tc.tile_pool(name="qtp", bufs=1))
            v_pool = ctx.enter_context(tc.tile_pool(name="vp", bufs=1))
            y_pool = ctx.enter_context(tc.tile_pool(name="yp", bufs=1))
            ex_pool = ctx.enter_context(tc.tile_pool(name="exp", bufs=10))
            rc_pool = ctx.enter_context(tc.tile_pool(name="rcp", bufs=2))
            rb_pool = ctx.enter_context(tc.tile_pool(name="rbp", bufs=2))
            yr_pool = ctx.enter_context(tc.tile_pool(name="yrp", bufs=2))
            ob_pool = ctx.enter_context(tc.tile_pool(name="ob", bufs=2))
            psS = ctx.enter_context(tc.tile_pool(name="psS", bufs=3, space="PSUM"))
            psY = ctx.enter_context(tc.tile_pool(name="psY", bufs=2, space="PSUM"))
            dram = ctx.enter_context(tc.tile_pool(name="dram", bufs=2,
                                                  space="DRAM"))

            # ---- DMAs in PE need-order, column-split so the first-half
            # tiles (all that phase-1/2 chains touch) land early
            bv_sb = consts.tile([P, FS], F32, tag="bv", name="bv_sb")
            nc.sync.dma_start(bv_sb[:], bv[:].to_broadcast((P, FS)))
            wvb = wv_pool.tile([P, NCB * FS], BF, tag="wv", name="wvb")
            nc.sync.dma_start(
                wvb[:].rearrange("p (cb f) -> p cb f", f=FS),
                wv[:].rearrange("(cb p) f -> p cb f", p=P),
            )
            wv_sb = [wvb[:, FS * cb : FS * (cb + 1)] for cb in range(NCB)]

            xtv = [xt_pool.tile([P, T], BF, tag="xt", name="xtv_t")
                   for cb in range(NCB)]
            for half in range(2):
                tsl = slice(1024 * half, 1024 * (half + 1))
                for cb in range(NCB):
                    nc.sync.dma_start(xtv[cb][:, tsl],
                                      xv[P * cb : P * (cb + 1), tsl])

            wkb = wqk_pool.tile([P, NFB * NCB * P], BF, tag="wkb", name="wkb")
            nc.sync.dma_start(
                wkb[:].rearrange("p (fb cb j) -> p fb cb j", cb=NCB, j=P), wk[:]
            )
            def wk_sb(fb, cb):
                o = NCB * P * fb + P * cb
                return wkb[:, o : o + P]

            ba_t = consts.tile([P, 2 * NFB + NCB], F32, tag="ba", name="ba_t")
            nc.sync.dma_start(ba_t[:, 0:NFB], bq[:])
            nc.sync.dma_start(ba_t[:, NFB : 2 * NFB], bk[:])
            nc.sync.dma_start(ba_t[:, 2 * NFB :], bo[:])
            bq_sb = [ba_t[:, i : i + 1] for i in range(NFB)]
            bk_sb = [ba_t[:, NFB + i : NFB + i + 1] for i in range(NFB)]
            bo_sb = [ba_t[:, 2 * NFB + i : 2 * NFB + i + 1] for i in range(NCB)]
            ones_sb = consts.tile([1, HD], BF, tag="ones", name="ones_sb")
            nc.gpsimd.memset(ones_sb[:], 1.0)

            xtk = [xt_pool.tile([P, T], BF, tag="xt", name="xtk_t")
                   for cb in range(NCB)]
            for cb in range(NCB):
                nc.sync.dma_start(xtk[cb][:, 0:1024],
                                  xk[P * cb : P * (cb + 1), 0:1024])

            wqb = wqk_pool.tile([P, NFB * NCB * P], BF, tag="wqb", name="wqb")
            nc.sync.dma_start(
                wqb[:].rearrange("p (fb cb j) -> p fb cb j", cb=NCB, j=P), wq[:]
            )
            def wq_sb(fb, cb):
                o = NCB * P * fb + P * cb
                return wqb[:, o : o + P]

            xtq = [xt_pool.tile([P, T], BF, tag="xt", name="xtq_t")
                   for cb in range(NCB)]
            for cb in range(NCB):
                nc.sync.dma_start(xtq[cb][:, 0:1024],
                                  xq[P * cb : P * (cb + 1), 0:1024])
            for cb in range(NCB):
                nc.sync.dma_start(xtk[cb][:, 1024:2048],
                                  xk[P * cb : P * (cb + 1), 1024:2048])
            for cb in range(NCB):
                nc.sync.dma_start(xtq[cb][:, 1024:2048],
                                  xq[P * cb : P * (cb + 1), 1024:2048])

            mx_sb = consts.tile([P, 896], BF, tag="maskx", name="mx_sb")
            nc.sync.dma_start(mx_sb[:], maskx[:])
            wob = wo_pool.tile([P, NCB * NFB * P], BF, tag="wo", name="wob")
            nc.sync.dma_start(
                wob[:].rearrange("p (cc fc j) -> p cc fc j", fc=NFB, j=P), wo[:]
            )
            def wo_sb(cc, fc):
                o = NFB * P * cc + P * fc
                return wob[:, o : o + P]

            # ---- persistent attention operands
            KT = [kt_pool.tile([P, T], BF, tag=f"kt{i}", name=f"kt{i}")
                  for i in range(NFB)]
            QT = [qt_pool.tile([P, T], BF, tag=f"qt{i}", name=f"qt{i}")
                  for i in range(NFB)]
            # V tiles carry an inline ones column per head: [v_h | 1] x 8
            VSB = [v_pool.tile([P, NH * (HD + 1)], BF, tag=f"v{i}", name=f"v{i}")
                   for i in range(NTT)]
            # Y stays resident in SBUF (f-major, head h rows [64h%128] of fb=h//2)
            YSB = [y_pool.tile([P, T], BF, tag=f"y{i}", name=f"y{i}")
                   for i in range(NFB)]

            # ==============  V projection (cb-major in 3-chain groups so the
            # PE has work while the xv tiles stream in)  ==============
            for tis in ([0, 1, 2], [3, 4, 5], [6, 7],
                        [8, 9, 10], [11, 12, 13], [14, 15]):
                pvs = {}
                for ti in tis:
                    pvs[ti] = psS.tile([P, FS], F32, tag="psS", name="pv")
                for cb in range(NCB):
                    for ti in tis:
                        nc.tensor.matmul(
                            pvs[ti][:], xtv[cb][:, P * ti : P * (ti + 1)],
                            wv_sb[cb],
                            start=(cb == 0), stop=(cb == NCB - 1),
                        )
                for ti in tis:
                    vt = VSB[ti]
                    v3 = vt[:].rearrange("p (h x) -> p h x", x=HD + 1)
                    nc.vector.tensor_add(
                        v3[:, :, 0:HD],
                        pvs[ti][:].rearrange("p (h d) -> p h d", d=HD),
                        bv_sb[:].rearrange("p (h d) -> p h d", d=HD),
                    )
                    nc.gpsimd.memset(v3[:, :, HD], 1.0)

            # ---- one K/Q projection chain: 1024 t-cols for one fb
            def proj_chain(fb, tqp, xt_src, w_sb, bias_sb, OUT):
                pp = psS.tile([P, 1024], F32, tag="psS", name="pp")
                for cb in range(NCB):
                    for u in range(2):
                        tq = 2 * tqp + u
                        nc.tensor.matmul(
                            pp[:, 512 * u : 512 * (u + 1)],
                            w_sb(fb, cb),
                            xt_src[cb][:, 512 * tq : 512 * (tq + 1)],
                            start=(cb == 0), stop=(cb == NCB - 1),
                        )
                for u in range(2):
                    tq = 2 * tqp + u
                    nc.vector.tensor_scalar_add(
                        OUT[fb][:, 512 * tq : 512 * (tq + 1)],
                        pp[:, 512 * u : 512 * (u + 1)],
                        bias_sb[fb],
                    )

            # half-size (512-col) projection chain: finer-grained filler
            def proj_chain_h(fb, tq, xt_src, w_sb, bias_sb, OUT):
                pp = psS.tile([P, 1024], F32, tag="psS", name="pph")
                for cb in range(NCB):
                    nc.tensor.matmul(
                        pp[:, 0:512],
                        w_sb(fb, cb),
                        xt_src[cb][:, 512 * tq : 512 * (tq + 1)],
                        start=(cb == 0), stop=(cb == NCB - 1),
                    )
                nc.vector.tensor_scalar_add(
                    OUT[fb][:, 512 * tq : 512 * (tq + 1)],
                    pp[:, 0:512],
                    bias_sb[fb],
                )

            # strip-0/1 K then Q projections up-front; tqp=1 chains become
            # attention-phase filler
            for fb in range(NFB):
                proj_chain(fb, 0, xtk, wk_sb, bk_sb, KT)
            for fb in range(NFB):
                proj_chain(fb, 0, xtq, wq_sb, bq_sb, QT)

            # ---- output projection chunk: 2 cc rows x one 512-col t-strip
            def outproj_chunk(tq, ccp, act_bias=False):
                pso = psS.tile([P, 1024], F32, tag="psS", name="pso")
                for fc in range(NFB):
                    for w in range(2):
                        cc = 2 * ccp + w
                        nc.tensor.matmul(
                            pso[:, 512 * w : 512 * (w + 1)],
                            wo_sb(cc, fc),
                            YSB[fc][:, 512 * tq : 512 * (tq + 1)],
                            start=(fc == 0), stop=(fc == NFB - 1),
                        )
                # host passes bo/2 so the host-side pair sum restores bo
                osb = ob_pool.tile([P, 1024], BF, tag="ob", name="osb")
                for w in range(2):
                    cc = 2 * ccp + w
                    if act_bias:
                        # tail strips: ACT is done with exps, so bias there
                        # keeps the (finalize-busy) DVE off this path
                        nc.scalar.add(
                            osb[:, 512 * w : 512 * (w + 1)],
                            pso[:, 512 * w : 512 * (w + 1)], bo_sb[cc])
                    else:
                        nc.vector.tensor_scalar_add(
                            osb[:, 512 * w : 512 * (w + 1)],
                            pso[:, 512 * w : 512 * (w + 1)], bo_sb[cc])
                    nc.sync.dma_start(
                        out[P * cc : P * (cc + 1),
                            512 * tq : 512 * (tq + 1)],
                        osb[:, 512 * w : 512 * (w + 1)],
                    )

            # ---- PE filler: K/Q strip-2/3 projection chunks are emitted
            # just-in-time per block; finished strips' output projections
            # queue up as generic filler.
            def k_chunk(tq, fb):
                proj_chain_h(fb, tq, xtk, wk_sb, bk_sb, KT)

            def q_chunk(tq, fb):
                proj_chain_h(fb, tq, xtq, wq_sb, bq_sb, QT)

            op_q = []

            def feed(n=1):
                for _ in range(min(n, len(op_q))):
                    op_q.pop(0)()

            # ================  attention: tq-outer, pair-inner  ================
            def attention_block(pair, tq, on_group, pe_bcast=False):
                ntk = 4 * (tq + 1)
                ngrp = ntk // 2
                qsl = slice(512 * tq, 512 * (tq + 1))
                psy = [
                    psY.tile([HD + 1, 512], F32, tag="psY", name=f"psy{s}")
                    for s in range(2)
                ]
                # software pipeline (depth 4): emit scores(g)+exp(g) ahead
                # of attV(g-4) so the PE's in-order stream never waits on
                # the ACT exp of the group it is about to consume.
                DEPTH = 4
                exq = {}
                for g in range(ngrp + DEPTH):
                    if g < ngrp:
                        for s in range(2):
                            rows = slice(64 * s, 64 * (s + 1))
                            ps = psS.tile([P, 1024], F32, tag="psS",
                                          name="ps_s")
                            for j in range(2):
                                tk = 2 * g + j
                                # diag tiles: only q >= 128*di is live
                                o_ = P * max(tk - 4 * tq, 0)
                                # 64-row array tiling: head s=0 in rows
                                # 0-63 (T0), s=1 in rows 64-127 (T8) so
                                # the two heads' LDW+MM run concurrently
                                nc.tensor.matmul(
                                    ps[:, 512 * j + o_ : 512 * (j + 1)],
                                    KT[pair][rows, P * tk : P * (tk + 1)],
                                    QT[pair][rows,
                                             512 * tq + o_ :
                                             512 * (tq + 1)],
                                    start=True, stop=True,
                                    tile_position=(64 * s, 0),
                                )
                            # exp; cols below the live offset hold garbage
                            # that no attV matmul reads. For the deep
                            # diagonal group, skip the dead columns.
                            ex = ex_pool.tile([P, 1024], BF, tag="ex",
                                              name="ex")
                            di0 = 2 * g - 4 * tq
                            if di0 == 2:
                                nc.scalar.activation(
                                    ex[:, 256:512], ps[:, 256:512],
                                    AF.Exp, scale=SCALE)
                                nc.scalar.activation(
                                    ex[:, 896:1024], ps[:, 896:1024],
                                    AF.Exp, scale=SCALE)
                            else:
                                nc.scalar.activation(ex[:], ps[:], AF.Exp,
                                                     scale=SCALE)
                            for j in range(2):
                                di = 2 * g + j - 4 * tq
                                if di >= 0:
                                    # triangular boundary block only
                                    o_ = 512 * j + P * di
                                    nc.vector.tensor_mul(
                                        ex[:, o_ : o_ + P],
                                        ex[:, o_ : o_ + P],
                                        mx_sb[:, 384:512],
                                    )
                            exq[(g, s)] = ex
                        on_group(g)
                    gd = g - DEPTH
                    if gd < 0:
                        continue
                    for s in range(2):
                        h = 2 * pair + s
                        vsl0 = (HD + 1) * h
                        ex = exq.pop((gd, s))
                        for j in range(2):
                            tk = 2 * gd + j
                            o_ = P * max(tk - 4 * tq, 0)
                            nc.tensor.matmul(
                                psy[s][:, o_:],
                                VSB[tk][:, vsl0 : vsl0 + HD + 1],
                                ex[:, 512 * j + o_ : 512 * (j + 1)],
                                start=(tk == 0), stop=(tk == ntk - 1),
                            )
                if pe_bcast:
                    # tail block: minimize finalize latency. Reciprocal
                    # straight off PSUM, y staged via the (now idle) ACT,
                    # and the partition-broadcast via a rank-1 ones matmul
                    # on the (otherwise idle) PE instead of the DRAM trip.
                    for s in range(2):
                        den = rc_pool.tile([1, 512], F32, tag="den",
                                           name="den")
                        nc.vector.tensor_copy(den[:], psy[s][HD : HD + 1, :])
                        rc = rc_pool.tile([1, 512], F32, tag="rc", name="rc")
                        nc.vector.reciprocal_approx_fast(rc[:], den[:])
                        rcb = rc_pool.tile([1, 512], BF, tag="rcb", name="rcb")
                        nc.vector.tensor_copy(rcb[:], rc[:])
                        yraw = yr_pool.tile([HD, 512], BF, tag="yr",
                                            name="yr")
                        nc.scalar.copy(yraw[:], psy[s][0:HD, :])
                        rbp = psS.tile([P, 1024], F32, tag="psS", name="rbp")
                        nc.tensor.matmul(rbp[0:HD, 0:512], ones_sb[:],
                                         rcb[:], start=True, stop=True)
                        nc.vector.tensor_mul(
                            YSB[pair][64 * s : 64 * (s + 1), qsl],
                            yraw[:], rbp[0:HD, 0:512],
                        )
                    return
                for s in range(2):
                    # stage y and denominator out of PSUM promptly so the
                    # psY slot frees for the next block
                    yraw = yr_pool.tile([HD, 512], BF, tag="yr", name="yr")
                    nc.vector.tensor_copy(yraw[:], psy[s][0:HD, :])
                    den = rc_pool.tile([1, 512], F32, tag="den", name="den")
                    nc.vector.tensor_copy(den[:], psy[s][HD : HD + 1, :])
                    rc = rc_pool.tile([1, 512], F32, tag="rc", name="rc")
                    nc.vector.reciprocal_approx_fast(rc[:], den[:])
                    # broadcast across partitions via a DRAM round-trip
                    # (partition-stride-0 DMA reads require a DRAM source);
                    # keeps the PE stream out of the finalize entirely
                    rcd = dram.tile([1, 512], F32, tag="rcd", name="rcd")
                    nc.sync.dma_start(rcd[:], rc[:])
                    rb = rb_pool.tile([HD, 512], F32, tag="rb", name="rb")
                    nc.sync.dma_start(rb[:], rcd[:].to_broadcast((HD, 512)))
                    nc.vector.tensor_mul(
                        YSB[pair][64 * s : 64 * (s + 1), qsl],
                        yraw[:], rb[:],
                    )

            # Strip-interleaved block order: heavy strips (2,3) alternate
            # with light strips (0,1) so the ACT exp stream stays fed from
            # the first scores to the last — otherwise the final strip's
            # exps pace the tail. K/Q column chunks for a heavy block are
            # emitted ~2 blocks ahead: their DVE bias-add needs slack
            # behind the finalize bursts or the scores stall on $S[DVE].
            BLOCK_ORDER = [(2, 0), (0, 0), (2, 1), (0, 1),
                           (2, 2), (0, 2), (2, 3), (0, 3),
                           (3, 0), (1, 0), (3, 1), (1, 1),
                           (3, 2), (1, 2), (3, 3), (1, 3)]
            # chunks emitted inside block idx (at groups 0,1) feed the
            # heavy block at idx+2
            PRE = {0: (2, 1), 2: (2, 2), 4: (2, 3), 6: (3, 0),
                   8: (3, 1), 10: (3, 2), 12: (3, 3)}
            q_chunk(2, 0)
            k_chunk(2, 0)
            strip_left = {tq: NFB for tq in range(NTQ)}
            for idx, (tq, pair) in enumerate(BLOCK_ORDER):
                ngrp = 2 * (tq + 1)
                last = (idx == len(BLOCK_ORDER) - 1)
                pre = PRE.get(idx)

                def on_group(g, ngrp=ngrp, idx=idx, last=last, pre=pre):
                    if pre is not None and g == 0:
                        q_chunk(*pre)
                    elif pre is not None and g == 1:
                        k_chunk(*pre)
                    elif last or (idx >= 8 and g in (2, ngrp // 2 + 2)):
                        feed(1)
                attention_block(pair, tq, on_group, pe_bcast=last)
                strip_left[tq] -= 1
                if strip_left[tq] == 0 and not last:
                    late = (tq == 3)
                    for ccp in range(NCB // 2):
                        op_q.append(
                            lambda tq=tq, ccp=ccp, late=late:
                                outproj_chunk(tq, ccp, act_bias=late))

            # drain leftover filler + last-finished strip's output projection
            feed(len(op_q))
            for ccp in range(NCB // 2):
                outproj_chunk(BLOCK_ORDER[-1][0], ccp, act_bias=True)

    nc.compile()
    return nc


_NC_CACHE = None


def _get_nc():
    global _NC_CACHE
    if _NC_CACHE is None:
        _NC_CACHE = build_program()
    return _NC_CACHE


def _host_consts():
    import ml_dtypes

    pgrid, ugrid = np.mgrid[0:P, 0:896]
    maskxv = (ugrid >= pgrid + 384).astype(ml_dtypes.bfloat16)
    return maskxv


def _w_qk_layout(w):
    # [p, fb, cb, j] = w[128*cb + p, 128*fb + j]
    return np.ascontiguousarray(
        w.reshape(NCB, P, NFB, P).transpose(1, 2, 0, 3))


def _w_o_layout(w):
    # [p, cc, fc, j] = w[128*fc + p, 128*cc + j]
    return np.ascontiguousarray(
        w.reshape(NFB, P, NCB, P).transpose(1, 2, 0, 3))


def _make_in_maps(inputs) -> list:
    import ml_dtypes

    BF16 = ml_dtypes.bfloat16

    def bf(a):
        return np.ascontiguousarray(np.asarray(a, dtype=np.float32)).astype(BF16)

    q = np.asarray(inputs["q"], dtype=np.float32)
    k = np.asarray(inputs["k"], dtype=np.float32)
    v = np.asarray(inputs["v"], dtype=np.float32)
    Wq = np.asarray(inputs["Wq"], dtype=np.float32)
    Wk = np.asarray(inputs["Wk"], dtype=np.float32)
    Wv = np.asarray(inputs["Wv"], dtype=np.float32)
    Wo = np.asarray(inputs["Wo"], dtype=np.float32)
    bq = np.asarray(inputs["bq"], dtype=np.float32)
    bk = np.asarray(inputs["bk"], dtype=np.float32)
    bv = np.asarray(inputs["bv"], dtype=np.float32)
    bo = np.asarray(inputs["bo"], dtype=np.float32)
    # mask is all-ones in this problem (causal handled in-kernel); ignored.

    maskxv = _host_consts()
    in_maps = []
    for c in range(NCORES):
        b, h2 = divmod(c, 2)
        fsl = slice(FS * h2, FS * (h2 + 1))
        in_maps.append({
            "xq": bf(q[b].T),
            "xk": bf(k[b].T),
            "xv": bf(v[b].T),
            "wq": _w_qk_layout(Wq[:, fsl]).astype(BF16),
            "wk": _w_qk_layout(Wk[:, fsl]).astype(BF16),
            "wv": bf(Wv[:, fsl]),
            "wo": _w_o_layout(Wo[fsl, :]).astype(BF16),
            "bq": np.ascontiguousarray(bq[fsl].reshape(NFB, P).T),
            "bk": np.ascontiguousarray(bk[fsl].reshape(NFB, P).T),
            "bv": np.ascontiguousarray(bv[fsl].reshape(1, FS)),
            "bo": np.ascontiguousarray((bo / 2.0).reshape(NCB, P).T),
            "maskx": maskxv,
        })
    return in_maps


def kernel(**inputs) -> np.ndarray:
    in_maps = _make_in_maps(inputs)
    nc = _get_nc()
    res = run_bass_kernel_spmd(nc, in_maps, list(range(NCORES)))

    full = np.empty((4, T, C), dtype=np.float32)
    for b in range(4):
        po = (res.results[2 * b]["out"].astype(np.float32)
              + res.results[2 * b + 1]["out"].astype(np.float32))
        full[b] = po.T
    return full


# revision 32
# speedup vs baseline: 1.0584x; 1.0304x over previous
"""Multi-head attention (B=4, T=2048, C=1024, H=16, causal) on 8 TRN2 cores.

Sharding: core c -> batch b = c//2, head-half h2 = c%2 (8 heads / core).
v4: host pre-transposes x (kills on-device xbar transposes), attention
runs tq-outer/pair-inner starting right after the strip-0 K/Q
projections, and the remaining projections + per-strip output
projection are interleaved into the attention phase as PE filler so
the exp-paced (ACT-saturated) window never idles the PE.
Each core emits its partial out^T over full T; the host sums the pair
during unshard (bo passed as bo/2).
"""

import sys

sys.path.insert(0, "/opt/trn_rl_repo")

import numpy as np

import concourse.bacc as bacc
import concourse.bass as bass
import concourse.mybir as mybir
import concourse.tile as tile
from concourse.bass_utils import run_bass_kernel_spmd

F32 = mybir.dt.float32
F32R = mybir.dt.float32r
BF = mybir.dt.bfloat16
AF = mybir.ActivationFunctionType

P = 128          # partitions
T = 2048         # sequence length
C = 1024         # model dim
FS = 512         # per-core feature slice (8 heads x 64)
NH = 8           # heads per core
HD = 64          # head dim
SCALE = 0.125    # 1/sqrt(64)
NCORES = 8

NTQ = 4          # T / 512 query tiles
NFB = 4          # FS / 128 feature blocks
NCB = 8          # C / 128 contraction blocks
NTT = 16         # T / 128 key tiles


def build_program():
    nc = bacc.Bacc(num_devices=NCORES)

    # host-pre-transposed inputs: x*[c, t] = x[t, c]
    xq = nc.declare_dram_parameter("xq", [C, T], BF, isOutput=False)
    xk = nc.declare_dram_parameter("xk", [C, T], BF, isOutput=False)
    xv = nc.declare_dram_parameter("xv", [C, T], BF, isOutput=False)
    # wq/wk[p, fb, cb, j] = W[128*cb + p, 512*h2 + 128*fb + j]
    wq = nc.declare_dram_parameter("wq", [P, NFB, NCB, P], BF, isOutput=False)
    wk = nc.declare_dram_parameter("wk", [P, NFB, NCB, P], BF, isOutput=False)
    # wv[p, cb*FS + f] = Wv[128*cb + p, fsl][f] (host pre-shuffled)
    wv = nc.declare_dram_parameter("wv", [P, NCB * FS], BF, isOutput=False)
    # wo[p, cc, fc, j] = Wo[fsl, :][128*fc + p, 128*cc + j]
    wo = nc.declare_dram_parameter("wo", [P, NCB, NFB, P], BF, isOutput=False)
    bq = nc.declare_dram_parameter("bq", [P, NFB], F32, isOutput=False)
    bk = nc.declare_dram_parameter("bk", [P, NFB], F32, isOutput=False)
    bv = nc.declare_dram_parameter("bv", [1, FS], F32, isOutput=False)
    bo = nc.declare_dram_parameter("bo", [P, NCB], F32, isOutput=False)
    # maskx[p, u] = 1.0 iff u >= p + 384; diag tile di mask slice at 384-128*di
    maskx = nc.declare_dram_parameter("maskx", [P, 896], BF, isOutput=False)
    out = nc.declare_dram_parameter("out", [C, T], BF, isOutput=True)

    with tile.TileContext(nc) as tc:
        import contextlib

        with contextlib.ExitStack() as ctx:
            consts = ctx.enter_context(tc.tile_pool(name="consts", bufs=1))
            xt_pool = ctx.enter_context(tc.tile_pool(name="xt", bufs=16))
            wqk_pool = ctx.enter_context(tc.tile_pool(name="wqk", bufs=1))
            wv_pool = ctx.enter_context(tc.tile_pool(name="wvp", bufs=1))
            wo_pool = ctx.enter_context(tc.tile_pool(name="wop", bufs=1))
            kt_pool = ctx.enter_context(tc.tile_pool(name="ktp", bufs=1))
            qt_pool = ctx.enter_context(tc.tile_pool(name="qtp", bufs=1))
            v_pool = ctx.enter_context(tc.tile_pool(name="vp", bufs=1))
            y_pool = ctx.enter_context(tc.tile_pool(name="yp", bufs=1))
            ex_pool = ctx.enter_context(tc.tile_pool(name="exp", bufs=10))
            rc_pool = ctx.enter_context(tc.tile_pool(name="rcp", bufs=2))
            rb_pool = ctx.enter_context(tc.tile_pool(name="rbp", bufs=2))
            yr_pool = ctx.enter_context(tc.tile_pool(name="yrp", bufs=2))
            ob_pool = ctx.enter_context(tc.tile_pool(name="ob", bufs=2))
            psS = ctx.enter_context(tc.tile_pool(name="psS", bufs=3, space="PSUM"))
            psY = ctx.enter_context(tc.tile_pool(name="psY", bufs=2, space="PSUM"))
            dram = ctx.enter_context(tc.tile_pool(name="dram", bufs=2,
                                                  space="DRAM"))

            # ---- DMAs in PE need-order, column-split so the first-half
            # tiles (all that phase-1/2 chains touch) land early
            bv_sb = consts.tile([P, FS], F32, tag="bv", name="bv_sb")
            nc.sync.dma_start(bv_sb[:], bv[:].to_broadcast((P, FS)))
            # wv arrives host-pre-shuffled to [p, cb*FS+f]: one contiguous DMA
            wvb = wv_pool.tile([P, NCB * FS], BF, tag="wv", name="wvb")
            nc.sync.dma_start(wvb[:], wv[:])
            wv_sb = [wvb[:, FS * cb : FS * (cb + 1)] for cb in range(NCB)]

            xtv = [xt_pool.tile([P, T], BF, tag="xt", name="xtv_t")
                   for cb in range(NCB)]
            for half in range(2):
                tsl = slice(1024 * half, 1024 * (half + 1))
                for cb in range(NCB):
                    nc.sync.dma_start(xtv[cb][:, tsl],
                                      xv[P * cb : P * (cb + 1), tsl])

            wkb = wqk_pool.tile([P, NFB * NCB * P], BF, tag="wkb", name="wkb")
            nc.sync.dma_start(
                wkb[:].rearrange("p (fb cb j) -> p fb cb j", cb=NCB, j=P), wk[:]
            )
            def wk_sb(fb, cb):
                o = NCB * P * fb + P * cb
                return wkb[:, o : o + P]

            ba_t = consts.tile([P, 2 * NFB + NCB], F32, tag="ba", name="ba_t")
            nc.sync.dma_start(ba_t[:, 0:NFB], bq[:])
            nc.sync.dma_start(ba_t[:, NFB : 2 * NFB], bk[:])
            nc.sync.dma_start(ba_t[:, 2 * NFB :], bo[:])
            bq_sb = [ba_t[:, i : i + 1] for i in range(NFB)]
            bk_sb = [ba_t[:, NFB + i : NFB + i + 1] for i in range(NFB)]
            bo_sb = [ba_t[:, 2 * NFB + i : 2 * NFB + i + 1] for i in range(NCB)]
            ones_sb = consts.tile([1, HD], BF, tag="ones", name="ones_sb")
            nc.gpsimd.memset(ones_sb[:], 1.0)

            xtk = [xt_pool.tile([P, T], BF, tag="xt", name="xtk_t")
                   for cb in range(NCB)]
            for cb in range(NCB):
                nc.sync.dma_start(xtk[cb][:, 0:1024],
                                  xk[P * cb : P * (cb + 1), 0:1024])

            wqb = wqk_pool.tile([P, NFB * NCB * P], BF, tag="wqb", name="wqb")
            nc.sync.dma_start(
                wqb[:].rearrange("p (fb cb j) -> p fb cb j", cb=NCB, j=P), wq[:]
            )
            def wq_sb(fb, cb):
                o = NCB * P * fb + P * cb
                return wqb[:, o : o + P]

            xtq = [xt_pool.tile([P, T], BF, tag="xt", name="xtq_t")
                   for cb in range(NCB)]
            for cb in range(NCB):
                nc.sync.dma_start(xtq[cb][:, 0:1024],
                                  xq[P * cb : P * (cb + 1), 0:1024])
            for cb in range(NCB):
                nc.sync.dma_start(xtk[cb][:, 1024:2048],
                                  xk[P * cb : P * (cb + 1), 1024:2048])
            for cb in range(NCB):
                nc.sync.dma_start(xtq[cb][:, 1024:2048],
                                  xq[P * cb : P * (cb + 1), 1024:2048])

            mx_sb = consts.tile([P, 896], BF, tag="maskx", name="mx_sb")
            nc.sync.dma_start(mx_sb[:], maskx[:])
            wob = wo_pool.tile([P, NCB * NFB * P], BF, tag="wo", name="wob")
            nc.sync.dma_start(
                wob[:].rearrange("p (cc fc j) -> p cc fc j", fc=NFB, j=P), wo[:]
            )
            def wo_sb(cc, fc):
                o = NFB * P * cc + P * fc
                return wob[:, o : o + P]

            # ---- persistent attention operands
            KT = [kt_pool.tile([P, T], BF, tag=f"kt{i}", name=f"kt{i}")
                  for i in range(NFB)]
            QT = [qt_pool.tile([P, T], BF, tag=f"qt{i}", name=f"qt{i}")
                  for i in range(NFB)]
            # V tiles carry an inline ones column per head: [v_h | 1] x 8
            VSB = [v_pool.tile([P, NH * (HD + 1)], BF, tag=f"v{i}", name=f"v{i}")
                   for i in range(NTT)]
            # Y stays resident in SBUF (f-major, head h rows [64h%128] of fb=h//2)
            YSB = [y_pool.tile([P, T], BF, tag=f"y{i}", name=f"y{i}")
                   for i in range(NFB)]

            # ==============  V projection (cb-major in 3-chain groups so the
            # PE has work while the xv tiles stream in)  ==============
            for tis in ([0, 1, 2], [3, 4, 5], [6, 7],
                        [8, 9, 10], [11, 12, 13], [14, 15]):
                pvs = {}
                for ti in tis:
                    pvs[ti] = psS.tile([P, FS], F32, tag="psS", name="pv")
                for cb in range(NCB):
                    for ti in tis:
                        nc.tensor.matmul(
                            pvs[ti][:], xtv[cb][:, P * ti : P * (ti + 1)],
                            wv_sb[cb],
                            start=(cb == 0), stop=(cb == NCB - 1),
                        )
                for ti in tis:
                    vt = VSB[ti]
                    v3 = vt[:].rearrange("p (h x) -> p h x", x=HD + 1)
                    nc.vector.tensor_add(
                        v3[:, :, 0:HD],
                        pvs[ti][:].rearrange("p (h d) -> p h d", d=HD),
                        bv_sb[:].rearrange("p (h d) -> p h d", d=HD),
                    )
                    nc.gpsimd.memset(v3[:, :, HD], 1.0)

            # ---- one K/Q projection chain: 1024 t-cols for one fb
            def proj_chain(fb, tqp, xt_src, w_sb, bias_sb, OUT):
                pp = psS.tile([P, 1024], F32, tag="psS", name="pp")
                for cb in range(NCB):
                    for u in range(2):
                        tq = 2 * tqp + u
                        nc.tensor.matmul(
                            pp[:, 512 * u : 512 * (u + 1)],
                            w_sb(fb, cb),
                            xt_src[cb][:, 512 * tq : 512 * (tq + 1)],
                            start=(cb == 0), stop=(cb == NCB - 1),
                        )
                for u in range(2):
                    tq = 2 * tqp + u
                    nc.vector.tensor_scalar_add(
                        OUT[fb][:, 512 * tq : 512 * (tq + 1)],
                        pp[:, 512 * u : 512 * (u + 1)],
                        bias_sb[fb],
                    )

            # half-size (512-col) projection chain: finer-grained filler
            def proj_chain_h(fb, tq, xt_src, w_sb, bias_sb, OUT):
                pp = psS.tile([P, 1024], F32, tag="psS", name="pph")
                for cb in range(NCB):
                    nc.tensor.matmul(
                        pp[:, 0:512],
                        w_sb(fb, cb),
                        xt_src[cb][:, 512 * tq : 512 * (tq + 1)],
                        start=(cb == 0), stop=(cb == NCB - 1),
                    )
                nc.vector.tensor_scalar_add(
                    OUT[fb][:, 512 * tq : 512 * (tq + 1)],
                    pp[:, 0:512],
                    bias_sb[fb],
                )

            # strip-0/1 K then Q projections up-front; tqp=1 chains become
            # attention-phase filler
            for fb in range(NFB):
                proj_chain(fb, 0, xtk, wk_sb, bk_sb, KT)
            for fb in range(NFB):
                proj_chain(fb, 0, xtq, wq_sb, bq_sb, QT)

            # ---- output projection chunk: 2 cc rows x one 512-col t-strip
            def outproj_chunk(tq, ccp, act_bias=False):
                pso = psS.tile([P, 1024], F32, tag="psS", name="pso")
                for fc in range(NFB):
                    for w in range(2):
                        cc = 2 * ccp + w
                        nc.tensor.matmul(
                            pso[:, 512 * w : 512 * (w + 1)],
                            wo_sb(cc, fc),
                            YSB[fc][:, 512 * tq : 512 * (tq + 1)],
                            start=(fc == 0), stop=(fc == NFB - 1),
                        )
                # host passes bo/2 so the host-side pair sum restores bo
                osb = ob_pool.tile([P, 1024], BF, tag="ob", name="osb")
                for w in range(2):
                    cc = 2 * ccp + w
                    if act_bias:
                        # tail strips: ACT is done with exps, so bias there
                        # keeps the (finalize-busy) DVE off this path
                        nc.scalar.add(
                            osb[:, 512 * w : 512 * (w + 1)],
                            pso[:, 512 * w : 512 * (w + 1)], bo_sb[cc])
                    else:
                        nc.vector.tensor_scalar_add(
                            osb[:, 512 * w : 512 * (w + 1)],
                            pso[:, 512 * w : 512 * (w + 1)], bo_sb[cc])
                    nc.sync.dma_start(
                        out[P * cc : P * (cc + 1),
                            512 * tq : 512 * (tq + 1)],
                        osb[:, 512 * w : 512 * (w + 1)],
                    )

            # ---- PE filler: K/Q strip-2/3 projection chunks are emitted
            # just-in-time per block; finished strips' output projections
            # queue up as generic filler.
            def k_chunk(tq, fb):
                proj_chain_h(fb, tq, xtk, wk_sb, bk_sb, KT)

            def q_chunk(tq, fb):
                proj_chain_h(fb, tq, xtq, wq_sb, bq_sb, QT)

            op_q = []

            def feed(n=1):
                for _ in range(min(n, len(op_q))):
                    op_q.pop(0)()

            # ================  attention: tq-outer, pair-inner  ================
            def attention_block(pair, tq, on_group, pe_bcast=False):
                ntk = 4 * (tq + 1)
                ngrp = ntk // 2
                qsl = slice(512 * tq, 512 * (tq + 1))
                psy = [
                    psY.tile([HD + 1, 512], F32, tag="psY", name=f"psy{s}")
                    for s in range(2)
                ]
                # software pipeline (depth 4): emit scores(g)+exp(g) ahead
                # of attV(g-4) so the PE's in-order stream never waits on
                # the ACT exp of the group it is about to consume.
                DEPTH = 4
                exq = {}
                for g in range(ngrp + DEPTH):
                    if g < ngrp:
                        for s in range(2):
                            rows = slice(64 * s, 64 * (s + 1))
                            ps = psS.tile([P, 1024], F32, tag="psS",
                                          name="ps_s")
                            for j in range(2):
                                tk = 2 * g + j
                                # diag tiles: only q >= 128*di is live
                                o_ = P * max(tk - 4 * tq, 0)
                                # 64-row array tiling: head s=0 in rows
                                # 0-63 (T0), s=1 in rows 64-127 (T8) so
                                # the two heads' LDW+MM run concurrently
                                nc.tensor.matmul(
                                    ps[:, 512 * j + o_ : 512 * (j + 1)],
                                    KT[pair][rows, P * tk : P * (tk + 1)],
                                    QT[pair][rows,
                                             512 * tq + o_ :
                                             512 * (tq + 1)],
                                    start=True, stop=True,
                                    tile_position=(64 * s, 0),
                                )
                            # exp; cols below the live offset hold garbage
                            # that no attV matmul reads. For the deep
                            # diagonal group, skip the dead columns.
                            ex = ex_pool.tile([P, 1024], BF, tag="ex",
                                              name="ex")
                            di0 = 2 * g - 4 * tq
                            if di0 == 2:
                                nc.scalar.activation(
                                    ex[:, 256:512], ps[:, 256:512],
                                    AF.Exp, scale=SCALE)
                                nc.scalar.activation(
                                    ex[:, 896:1024], ps[:, 896:1024],
                                    AF.Exp, scale=SCALE)
                            else:
                                nc.scalar.activation(ex[:], ps[:], AF.Exp,
                                                     scale=SCALE)
                            for j in range(2):
                                di = 2 * g + j - 4 * tq
                                if di >= 0:
                                    # triangular boundary block only
                                    o_ = 512 * j + P * di
                                    nc.vector.tensor_mul(
                                        ex[:, o_ : o_ + P],
                                        ex[:, o_ : o_ + P],
                                        mx_sb[:, 384:512],
                                    )
                            exq[(g, s)] = ex
                        on_group(g)
                    gd = g - DEPTH
                    if gd < 0:
                        continue
                    for s in range(2):
                        h = 2 * pair + s
                        vsl0 = (HD + 1) * h
                        ex = exq.pop((gd, s))
                        for j in range(2):
                            tk = 2 * gd + j
                            o_ = P * max(tk - 4 * tq, 0)
                            nc.tensor.matmul(
                                psy[s][:, o_:],
                                VSB[tk][:, vsl0 : vsl0 + HD + 1],
                                ex[:, 512 * j + o_ : 512 * (j + 1)],
                                start=(tk == 0), stop=(tk == ntk - 1),
                            )
                if pe_bcast:
                    # tail block: minimize finalize latency. Reciprocal
                    # straight off PSUM, y staged via the (now idle) ACT,
                    # and the partition-broadcast via a rank-1 ones matmul
                    # on the (otherwise idle) PE instead of the DRAM trip.
                    for s in range(2):
                        den = rc_pool.tile([1, 512], F32, tag="den",
                                           name="den")
                        nc.vector.tensor_copy(den[:], psy[s][HD : HD + 1, :])
                        rc = rc_pool.tile([1, 512], F32, tag="rc", name="rc")
                        nc.vector.reciprocal_approx_fast(rc[:], den[:])
                        rcb = rc_pool.tile([1, 512], BF, tag="rcb", name="rcb")
                        nc.vector.tensor_copy(rcb[:], rc[:])
                        yraw = yr_pool.tile([HD, 512], BF, tag="yr",
                                            name="yr")
                        nc.scalar.copy(yraw[:], psy[s][0:HD, :])
                        rbp = psS.tile([P, 1024], F32, tag="psS", name="rbp")
                        nc.tensor.matmul(rbp[0:HD, 0:512], ones_sb[:],
                                         rcb[:], start=True, stop=True)
                        nc.vector.tensor_mul(
                            YSB[pair][64 * s : 64 * (s + 1), qsl],
                            yraw[:], rbp[0:HD, 0:512],
                        )
                    return
                for s in range(2):
                    # stage y and denominator out of PSUM promptly so the
                    # psY slot frees for the next block
                    yraw = yr_pool.tile([HD, 512], BF, tag="yr", name="yr")
                    nc.vector.tensor_copy(yraw[:], psy[s][0:HD, :])
                    den = rc_pool.tile([1, 512], F32, tag="den", name="den")
                    nc.vector.tensor_copy(den[:], psy[s][HD : HD + 1, :])
                    rc = rc_pool.tile([1, 512], F32, tag="rc", name="rc")
                    nc.vector.reciprocal_approx_fast(rc[:], den[:])
                    # broadcast across partitions via a DRAM round-trip
                    # (partition-stride-0 DMA reads require a DRAM source);
                    # keeps the PE stream out of the finalize entirely
                    rcd = dram.tile([1, 512], F32, tag="rcd", name="rcd")
                    nc.sync.dma_start(rcd[:], rc[:])
                    rb = rb_pool.tile([HD, 512], F32, tag="rb", name="rb")
                    nc.sync.dma_start(rb[:], rcd[:].to_broadcast((HD, 512)))
                    nc.vector.tensor_mul(
                        YSB[pair][64 * s : 64 * (s + 1), qsl],
                        yraw[:], rb[:],
                    )

            # Sequential strips with budgeted filler feeds; block (1,3) is
            # deferred to the very end so strip-3's final exps are consumed
            # while the PE runs that light block plus strip-3's output
            # projection, instead of pacing an idle tail.
            proj_q = []
            for fb in range(NFB):
                proj_q.append(lambda fb=fb: k_chunk(2, fb))
                proj_q.append(lambda fb=fb: q_chunk(2, fb))
            for fb in range(NFB):
                proj_q.append(lambda fb=fb: k_chunk(3, fb))
                proj_q.append(lambda fb=fb: q_chunk(3, fb))

            def feed_any(n=1):
                for _ in range(n):
                    if proj_q:
                        proj_q.pop(0)()
                    elif op_q:
                        op_q.pop(0)()
                    else:
                        break

            BLOCK_SEQ = ([(0, p) for p in range(NFB)]
                         + [(1, p) for p in range(NFB - 1)]
                         + [(2, p) for p in range(NFB)]
                         + [(3, p) for p in range(NFB)])
            ALLOW = {0: 1, 1: 4, 2: 1, 3: 2}
            for tq, pair in BLOCK_SEQ:
                if tq == 2:
                    while proj_q:
                        feed_any(1)
                ngrp = 2 * (tq + 1)
                a = min(ALLOW[tq], ngrp)
                slots = {(i * ngrp) // a for i in range(a)}

                def on_group(g, slots=slots):
                    if g in slots:
                        feed_any(1)
                attention_block(pair, tq, on_group)
                if (tq, pair) == (0, NFB - 1):
                    for ccp in range(NCB // 2):
                        op_q.append(
                            lambda ccp=ccp: outproj_chunk(0, ccp))
                elif (tq, pair) == (2, NFB - 1):
                    for ccp in range(NCB // 2):
                        op_q.append(
                            lambda ccp=ccp: outproj_chunk(2, ccp))
                elif (tq, pair) == (3, NFB - 1):
                    for ccp in range(NCB // 2):
                        op_q.append(
                            lambda ccp=ccp:
                                outproj_chunk(3, ccp, act_bias=True))

            # final light block; strip-3 outproj chunks feed its groups
            def on_group_last(g):
                feed_any(1)
            attention_block(NFB - 1, 1, on_group_last, pe_bcast=True)
            feed_any(len(proj_q) + len(op_q))
            for ccp in range(NCB // 2):
                outproj_chunk(1, ccp, act_bias=True)

    nc.compile()
    return nc


_NC_CACHE = None


def _get_nc():
    global _NC_CACHE
    if _NC_CACHE is None:
        _NC_CACHE = build_program()
    return _NC_CACHE


def _host_consts():
    import ml_dtypes

    pgrid, ugrid = np.mgrid[0:P, 0:896]
    maskxv = (ugrid >= pgrid + 384).astype(ml_dtypes.bfloat16)
    return maskxv


def _w_qk_layout(w):
    # [p, fb, cb, j] = w[128*cb + p, 128*fb + j]
    return np.ascontiguousarray(
        w.reshape(NCB, P, NFB, P).transpose(1, 2, 0, 3))


def _w_o_layout(w):
    # [p, cc, fc, j] = w[128*fc + p, 128*cc + j]
    return np.ascontiguousarray(
        w.reshape(NFB, P, NCB, P).transpose(1, 2, 0, 3))


def _make_in_maps(inputs) -> list:
    import ml_dtypes

    BF16 = ml_dtypes.bfloat16

    def bf(a):
        return np.ascontiguousarray(np.asarray(a, dtype=np.float32)).astype(BF16)

    q = np.asarray(inputs["q"], dtype=np.float32)
    k = np.asarray(inputs["k"], dtype=np.float32)
    v = np.asarray(inputs["v"], dtype=np.float32)
    Wq = np.asarray(inputs["Wq"], dtype=np.float32)
    Wk = np.asarray(inputs["Wk"], dtype=np.float32)
    Wv = np.asarray(inputs["Wv"], dtype=np.float32)
    Wo = np.asarray(inputs["Wo"], dtype=np.float32)
    bq = np.asarray(inputs["bq"], dtype=np.float32)
    bk = np.asarray(inputs["bk"], dtype=np.float32)
    bv = np.asarray(inputs["bv"], dtype=np.float32)
    bo = np.asarray(inputs["bo"], dtype=np.float32)
    # mask is all-ones in this problem (causal handled in-kernel); ignored.

    maskxv = _host_consts()
    in_maps = []
    for c in range(NCORES):
        b, h2 = divmod(c, 2)
        fsl = slice(FS * h2, FS * (h2 + 1))
        in_maps.append({
            "xq": bf(q[b].T),
            "xk": bf(k[b].T),
            "xv": bf(v[b].T),
            "wq": _w_qk_layout(Wq[:, fsl]).astype(BF16),
            "wk": _w_qk_layout(Wk[:, fsl]).astype(BF16),
            "wv": np.ascontiguousarray(
                bf(Wv[:, fsl]).reshape(NCB, P, FS).transpose(1, 0, 2)
                .reshape(P, NCB * FS)),
            "wo": _w_o_layout(Wo[fsl, :]).astype(BF16),
            "bq": np.ascontiguousarray(bq[fsl].reshape(NFB, P).T),
            "bk": np.ascontiguousarray(bk[fsl].reshape(NFB, P).T),
            "bv": np.ascontiguousarray(bv[fsl].reshape(1, FS)),
            "bo": np.ascontiguousarray((bo / 2.0).reshape(NCB, P).T),
            "maskx": maskxv,
        })
    return in_maps


def kernel(**inputs) -> np.ndarray:
    in_maps = _make_in_maps(inputs)
    nc = _get_nc()
    res = run_bass_kernel_spmd(nc, in_maps, list(range(NCORES)))

    full = np.empty((4, T, C), dtype=np.float32)
    for b in range(4):
        po = (res.results[2 * b]["out"].astype(np.float32)
              + res.results[2 * b + 1]["out"].astype(np.float32))
        full[b] = po.T
    return full
